# revision 14
# baseline (speedup 1.0000x reference)
"""Trainium2 Bass kernel for nn_EventTemplateBank (batched 1-D template-bank conv).

Math: score[b,t,e] = sum_{f,l} delayed[e,f,l] * x[b, t+40-l, f] / (L*F),
with delayed = delay-shifted templates (zero fill) and x zero-padded.

Device formulation (per core, data-parallel over batch):
  - Interleaved-slot contraction: the 128-position x-window of an output
    column is stored feature-interleaved as 768 flat slots
    (slot = 6*window_pos + feature), chunked into 6 tiles of 128
    partitions. Each PSUM set s (outputs t = 48n + 8s + dd, dd in [0,8))
    needs taps spanning 522 consecutive slots, which always fits in 5
    consecutive tiles: sets 0-2 use tiles 0-4, sets 3-5 use tiles 1-5.
    30 matmuls per 512-column block instead of the 36 a per-feature
    window layout needs (75% vs 62.5% PE efficiency).
  - x scratch is block-contiguous per partition: one DMA per 512-column
    block moves 128 descriptors of 6KB (not 768 of 1KB), so SWDGE
    descriptor generation stays far ahead of the PE.
  - Everything bf16 (x scratch, Toeplitz weights, output): halves HBM
    traffic vs fp32; PSUM accumulation is fp32.
  - Output written to DRAM in matmul-native layout; host re-permutes.
"""

import numpy as np
import ml_dtypes

import concourse.mybir as mybir
from concourse import bacc
from concourse.bass_utils import run_bass_kernel_spmd
from concourse.tile import TileContext

# Problem shapes (hardcoded per contract)
B, S, F = 64, 32768, 6
E, L = 16, 80
MAX_DELAY = 10

NCORES = 8
BPC = B // NCORES          # batches per core
Q = 48                     # output positions per rhs column
KWIN = 128                 # window positions per column
NTILE = 6                  # 768 slots = 6 tiles of 128 partitions
TPS = 5                    # tiles per set (522 slots span 5 tiles)
NS = 6                     # d-sets of 8 -> D in [0, 48)
PADF = 39                  # window of column n starts at 48n - 39
NCOLB = (S + Q - 1) // Q   # 683 columns per batch
BLKN = 512                 # columns per matmul block
NBLK = 11                  # ceil(8*683 / 512)
CPAD = NBLK * BLKN         # 5632 padded columns per core
CTOT = BPC * NCOLB         # 5464 real columns per core
LASTN = CTOT - (NBLK - 1) * BLKN   # 344 real columns in the last block

BF16 = ml_dtypes.bfloat16
LAST_RESULT = None         # BassKernelResults of the most recent run (for profiling)


def _tile_of(s: int, tl: int) -> int:
    return tl + (1 if s >= 3 else 0)


def _build_weights(templates: np.ndarray, onset_delays: np.ndarray) -> np.ndarray:
    """W[p, s, tl, 16dd+e] = delayed[e, f, (8s+dd)+79-k] / (L*F), zero outside [0,L),
    where (k, f) = divmod(128*tile_of(s,tl) + p, 6)."""
    d = np.round(np.clip(onset_delays, -MAX_DELAY, MAX_DELAY)).astype(np.int64)
    idx = np.arange(L)
    src = idx[None, None, :] - d[:, :, None]                 # (E,F,L)
    valid = (src >= 0) & (src < L)
    delayed = np.take_along_axis(templates, np.clip(src, 0, L - 1), axis=2)
    delayed = np.where(valid, delayed, 0.0).astype(np.float32) / float(L * F)

    W = np.zeros((KWIN, NS, TPS, 128), dtype=np.float32)
    dd = np.arange(8)
    for s in range(NS):
        for tl in range(TPS):
            slot = 128 * _tile_of(s, tl) + np.arange(128)
            k = slot // F
            f = slot % F
            l = (8 * s + dd)[None, :] + 79 - k[:, None]      # (128, 8)
            ok = (l >= 0) & (l < L)
            g = delayed[:, f[:, None], np.clip(l, 0, L - 1)]  # (E, 128, 8)
            g = np.where(ok[None], g, 0.0)
            W[:, s, tl, :] = g.transpose(1, 2, 0).reshape(128, 128)
    return np.ascontiguousarray(W.astype(BF16))


def _build_xsc(x: np.ndarray) -> np.ndarray:
    """Xsc[core, blk, p, t, c] = xflat[b, 288*n - 234 + 128*t + p], n = blk*512 + c
    mapped per batch (683 columns each), block-contiguous per partition."""
    need = Q * (NCOLB - 1) + KWIN
    xpad = np.zeros((B, PADF + need, F), dtype=BF16)
    xpad[:, PADF:PADF + S, :] = x.astype(BF16)
    xflat = np.ascontiguousarray(xpad.reshape(B, -1))
    ez = xflat.strides[1]
    v = np.lib.stride_tricks.as_strided(
        xflat, shape=(B, NTILE, KWIN, NCOLB),
        strides=(xflat.strides[0], 128 * ez, ez, Q * F * ez),
    )
    arr = np.zeros((NCORES, NTILE, KWIN, CPAD), dtype=BF16)
    for b in range(B):
        core, i = divmod(b, BPC)
        arr[core, :, :, i * NCOLB:(i + 1) * NCOLB] = v[b]
    # (core, t, p, blk, c) -> (core, blk, p, t, c): per partition, one 6KB run per block
    arr = arr.reshape(NCORES, NTILE, KWIN, NBLK, BLKN).transpose(0, 3, 2, 1, 4)
    return np.ascontiguousarray(arr)


def _build_program():
    f32 = mybir.dt.float32
    bf16 = mybir.dt.bfloat16
    nc = bacc.Bacc("TRN2", target_bir_lowering=False, debug=False)
    xsc = nc.dram_tensor("xsc", [NBLK, KWIN, NTILE * BLKN], bf16, kind="ExternalInput")
    w = nc.dram_tensor("w", [KWIN, NS, TPS, 128], bf16, kind="ExternalInput")
    osc = nc.dram_tensor("osc", [NBLK, KWIN, NS, BLKN], bf16, kind="ExternalOutput")

    with TileContext(nc) as tc:
        with (
            tc.tile_pool(name="wp", bufs=1) as wp,
            tc.tile_pool(name="xp", bufs=6) as xp,
            tc.tile_pool(name="pp", bufs=8, space="PSUM") as pp,
            tc.tile_pool(name="op", bufs=3) as op,
        ):
            # Startup critical path: the scalar HWDGE ring is the first
            # sequencer free after the preamble, so it carries block-0's
            # first x tiles interleaved with the weight chunks in exactly
            # the order the tile-major block-0 matmuls consume them.
            # Block-0's trailing tiles ride the sync ring concurrently;
            # steady-state blocks use one SWDGE DMA each (128 x 6KB).
            wt = wp.tile([KWIN, NS * TPS * 128], bf16)
            wr = w.rearrange("p s t m -> p (s t m)")

            def wdma(s):
                sl = slice(s * TPS * 128, (s + 1) * TPS * 128)
                nc.scalar.dma_start(out=wt[:, sl], in_=wr[:, sl])

            PREFETCH = 3
            xtiles = {}

            def issue_x(blk):
                xf = xp.tile([KWIN, NTILE * BLKN], bf16, tag="xtp")
                if blk == 0:
                    for t in range(3):
                        nc.scalar.dma_start(
                            out=xf[:, t * BLKN:(t + 1) * BLKN],
                            in_=xsc[blk, :, t * BLKN:(t + 1) * BLKN],
                        )
                        wdma(t)          # x-t0, w-s0, x-t1, w-s1, x-t2, w-s2
                    for t in range(3, NTILE):
                        nc.sync.dma_start(
                            out=xf[:, t * BLKN:(t + 1) * BLKN],
                            in_=xsc[blk, :, t * BLKN:(t + 1) * BLKN],
                        )
                    for s in range(3, NS):
                        wdma(s)
                else:
                    nc.gpsimd.dma_start(out=xf, in_=xsc[blk])
                xtiles[blk] = xf

            for blk in range(min(PREFETCH + 1, NBLK)):
                issue_x(blk)

            for blk in range(NBLK):
                n = BLKN if blk < NBLK - 1 else LASTN
                if blk + PREFETCH + 1 < NBLK:
                    issue_x(blk + PREFETCH + 1)
                xf = xtiles.pop(blk)
                pss = [
                    pp.tile([128, n], f32, tag="ps", name=f"ps_{blk}_{s}")
                    for s in range(NS)
                ]
                obt = op.tile([KWIN, NS, BLKN], bf16, tag="ot", name=f"ot_{blk}")
                last = blk == NBLK - 1
                if last:
                    otl = op.tile([KWIN, BLKN], bf16, tag="otl", name="otl")
                else:
                    otl = None

                def evac(s, n=n, pss=pss, obt=obt, otl=otl, last=last):
                    # alternate Vector/Scalar so PSUM banks recycle 2x faster
                    eng = nc.vector.tensor_copy if s % 2 == 0 else nc.scalar.copy
                    if last and s == NS - 1:
                        eng(out=otl[:, 0:n], in_=pss[s])
                    else:
                        eng(out=obt[:, s, 0:n], in_=pss[s])

                if blk == 0:
                    # tile-major: each arriving x tile feeds every set using
                    # it, so the PE starts as soon as tile 0 lands.
                    for t in range(NTILE):
                        for s in range(NS):
                            tl = t - (1 if s >= 3 else 0)
                            if 0 <= tl < TPS:
                                nc.tensor.matmul(
                                    pss[s],
                                    wt[:, (s * TPS + tl) * 128:(s * TPS + tl + 1) * 128],
                                    xf[:, t * BLKN:t * BLKN + n],
                                    start=(tl == 0),
                                    stop=(tl == TPS - 1),
                                    skip_group_check=True,
                                )
                    for s in range(NS):
                        evac(s)
                else:
                    for s in range(NS):
                        g = 1 if s >= 3 else 0
                        for tl in range(TPS):
                            T = tl + g
                            nc.tensor.matmul(
                                pss[s],
                                wt[:, (s * TPS + tl) * 128:(s * TPS + tl + 1) * 128],
                                xf[:, T * BLKN:T * BLKN + n],
                                start=(tl == 0),
                                stop=(tl == TPS - 1),
                            )
                        evac(s)
                # one output DMA per block (128 descriptors of 6KB); the last
                # block splits off set 5 so the kernel tail only waits on 88KB
                if not last:
                    nc.sync.dma_start(out=osc[blk, :, :, 0:n], in_=obt[:, :, 0:n])
                else:
                    nc.sync.dma_start(out=osc[blk, :, 0:NS - 1, 0:n], in_=obt[:, 0:NS - 1, 0:n])
                    nc.sync.dma_start(out=osc[blk, :, NS - 1, 0:n], in_=otl[:, 0:n])
    nc.compile()
    return nc


def kernel(x: np.ndarray, templates: np.ndarray, onset_delays: np.ndarray) -> np.ndarray:
    global LAST_RESULT
    x = np.ascontiguousarray(x, dtype=np.float32)
    templates = np.asarray(templates, dtype=np.float32)
    onset_delays = np.asarray(onset_delays, dtype=np.float32)

    W = _build_weights(templates, onset_delays)
    Xsc = _build_xsc(x)                                   # (NCORES, NBLK, K, NTILE*BLKN)

    nc = _build_program()
    in_maps = [{"xsc": Xsc[c], "w": W} for c in range(NCORES)]
    res = run_bass_kernel_spmd(nc, in_maps, core_ids=list(range(NCORES)))
    LAST_RESULT = res

    osc = np.stack([np.asarray(r["osc"]) for r in res.results], axis=0)
    o = osc.astype(np.float32)                                 # (NCORES,NBLK,128,NS,BLKN)
    o = o.reshape(NCORES, NBLK, 8, E, NS, BLKN)                # core, blk, dd, e, s, n
    o = o.transpose(0, 1, 5, 4, 2, 3)                          # core, blk, n, s, dd, e
    o = np.ascontiguousarray(o).reshape(NCORES, CPAD, NS * 8 * E)
    o = o[:, :BPC * NCOLB, :].reshape(NCORES, BPC, NCOLB, NS, 8, E)
    o = o.reshape(B, NCOLB * Q, E)[:, :S, :]
    o = np.ascontiguousarray(o)
    o[:, S - 1, :] = 0.0                                   # reference zero-pads last column
    return o


# revision 15
# speedup vs baseline: 1.0943x; 1.0943x over previous
"""Trainium2 Bass kernel for nn_EventTemplateBank (batched 1-D template-bank conv).

Math: score[b,t,e] = sum_{f,l} delayed[e,f,l] * x[b, t+40-l, f] / (L*F),
with delayed = delay-shifted templates (zero fill) and x zero-padded.

Device formulation (per core, data-parallel over batch):
  - Interleaved-slot contraction: the 128-position x-window of an output
    column is stored feature-interleaved as 768 flat slots
    (slot = 6*window_pos + feature), chunked into 6 tiles of 128
    partitions. Each PSUM set s (outputs t = 48n + 8s + dd, dd in [0,8))
    needs taps spanning 522 consecutive slots, which always fits in 5
    consecutive tiles: sets 0-2 use tiles 0-4, sets 3-5 use tiles 1-5.
    30 matmuls per 512-column block instead of the 36 a per-feature
    window layout needs (75% vs 62.5% PE efficiency).
  - x scratch is block-contiguous per partition: one DMA per 512-column
    block moves 128 descriptors of 6KB (not 768 of 1KB), so SWDGE
    descriptor generation stays far ahead of the PE.
  - Everything bf16 (x scratch, Toeplitz weights, output): halves HBM
    traffic vs fp32; PSUM accumulation is fp32.
  - Output written to DRAM in matmul-native layout; host re-permutes.
"""

import numpy as np
import ml_dtypes

import concourse.mybir as mybir
from concourse import bacc
from concourse.bass_utils import run_bass_kernel_spmd
from concourse.tile import TileContext

# Problem shapes (hardcoded per contract)
B, S, F = 64, 32768, 6
E, L = 16, 80
MAX_DELAY = 10

NCORES = 8
BPC = B // NCORES          # batches per core
Q = 48                     # output positions per rhs column
KWIN = 128                 # window positions per column
NTILE = 6                  # 768 slots = 6 tiles of 128 partitions
TPS = 5                    # tiles per set (522 slots span 5 tiles)
NS = 6                     # d-sets of 8 -> D in [0, 48)
PADF = 39                  # window of column n starts at 48n - 39
NCOLB = (S + Q - 1) // Q   # 683 columns per batch
BLKN = 512                 # columns per matmul block
NBLK = 11                  # ceil(8*683 / 512)
CPAD = NBLK * BLKN         # 5632 padded columns per core
CTOT = BPC * NCOLB         # 5464 real columns per core
LASTN = CTOT - (NBLK - 1) * BLKN   # 344 real columns in the last block

BF16 = ml_dtypes.bfloat16
LAST_RESULT = None         # BassKernelResults of the most recent run (for profiling)


def _tile_of(s: int, tl: int) -> int:
    return tl + (1 if s >= 3 else 0)


def _build_weights(templates: np.ndarray, onset_delays: np.ndarray) -> np.ndarray:
    """W[p, s, tl, 16dd+e] = delayed[e, f, (8s+dd)+79-k] / (L*F), zero outside [0,L),
    where (k, f) = divmod(128*tile_of(s,tl) + p, 6)."""
    d = np.round(np.clip(onset_delays, -MAX_DELAY, MAX_DELAY)).astype(np.int64)
    idx = np.arange(L)
    src = idx[None, None, :] - d[:, :, None]                 # (E,F,L)
    valid = (src >= 0) & (src < L)
    delayed = np.take_along_axis(templates, np.clip(src, 0, L - 1), axis=2)
    delayed = np.where(valid, delayed, 0.0).astype(np.float32) / float(L * F)

    W = np.zeros((KWIN, NS, TPS, 128), dtype=np.float32)
    dd = np.arange(8)
    for s in range(NS):
        for tl in range(TPS):
            slot = 128 * _tile_of(s, tl) + np.arange(128)
            k = slot // F
            f = slot % F
            l = (8 * s + dd)[None, :] + 79 - k[:, None]      # (128, 8)
            ok = (l >= 0) & (l < L)
            g = delayed[:, f[:, None], np.clip(l, 0, L - 1)]  # (E, 128, 8)
            g = np.where(ok[None], g, 0.0)
            W[:, s, tl, :] = g.transpose(1, 2, 0).reshape(128, 128)
    return np.ascontiguousarray(W.astype(BF16))


def _build_xsc(x: np.ndarray) -> np.ndarray:
    """Xsc[core, blk, p, t, c] = xflat[b, 288*n - 234 + 128*t + p], n = blk*512 + c
    mapped per batch (683 columns each), block-contiguous per partition."""
    need = Q * (NCOLB - 1) + KWIN
    xpad = np.zeros((B, PADF + need, F), dtype=BF16)
    xpad[:, PADF:PADF + S, :] = x.astype(BF16)
    xflat = np.ascontiguousarray(xpad.reshape(B, -1))
    ez = xflat.strides[1]
    v = np.lib.stride_tricks.as_strided(
        xflat, shape=(B, NTILE, KWIN, NCOLB),
        strides=(xflat.strides[0], 128 * ez, ez, Q * F * ez),
    )
    arr = np.zeros((NCORES, NTILE, KWIN, CPAD), dtype=BF16)
    for b in range(B):
        core, i = divmod(b, BPC)
        arr[core, :, :, i * NCOLB:(i + 1) * NCOLB] = v[b]
    # (core, t, p, blk, c) -> (core, blk, p, t, c): per partition, one 6KB run per block
    arr = arr.reshape(NCORES, NTILE, KWIN, NBLK, BLKN).transpose(0, 3, 2, 1, 4)
    return np.ascontiguousarray(arr)


def _build_program():
    f32 = mybir.dt.float32
    bf16 = mybir.dt.bfloat16
    nc = bacc.Bacc("TRN2", target_bir_lowering=False, debug=False)
    xsc = nc.dram_tensor("xsc", [NBLK, KWIN, NTILE * BLKN], bf16, kind="ExternalInput")
    w = nc.dram_tensor("w", [KWIN, NS, TPS, 128], bf16, kind="ExternalInput")
    osc = nc.dram_tensor("osc", [NBLK, KWIN, NS, BLKN], bf16, kind="ExternalOutput")

    with TileContext(nc) as tc:
        with (
            tc.tile_pool(name="wp", bufs=1) as wp,
            tc.tile_pool(name="xp", bufs=6) as xp,
            tc.tile_pool(name="pp", bufs=8, space="PSUM") as pp,
            tc.tile_pool(name="op", bufs=3) as op,
        ):
            # Startup critical path: the scalar HWDGE ring is the first
            # sequencer free after the preamble, so it carries block-0's
            # first x tiles interleaved with the weight chunks in exactly
            # the order the tile-major block-0 matmuls consume them.
            # Block-0's trailing tiles ride the sync ring concurrently;
            # steady-state blocks use one SWDGE DMA each (128 x 6KB).
            wt = wp.tile([KWIN, NS * TPS * 128], bf16)
            wr = w.rearrange("p s t m -> p (s t m)")

            def wdma(s):
                sl = slice(s * TPS * 128, (s + 1) * TPS * 128)
                nc.scalar.dma_start(out=wt[:, sl], in_=wr[:, sl])

            PREFETCH = 3
            xtiles = {}

            for s in range(NS):
                wdma(s)

            def issue_x(blk):
                xf = xp.tile([KWIN, NTILE * BLKN], bf16, tag="xtp")
                if blk == 0:
                    for t in range(NTILE):
                        nc.gpsimd.dma_start(
                            out=xf[:, t * BLKN:(t + 1) * BLKN],
                            in_=xsc[blk, :, t * BLKN:(t + 1) * BLKN],
                        )
                else:
                    nc.gpsimd.dma_start(out=xf, in_=xsc[blk])
                xtiles[blk] = xf

            for blk in range(min(PREFETCH + 1, NBLK)):
                issue_x(blk)

            for blk in range(NBLK):
                n = BLKN if blk < NBLK - 1 else LASTN
                if blk + PREFETCH + 1 < NBLK:
                    issue_x(blk + PREFETCH + 1)
                xf = xtiles.pop(blk)
                pss = [
                    pp.tile([128, n], f32, tag="ps", name=f"ps_{blk}_{s}")
                    for s in range(NS)
                ]
                obt = op.tile([KWIN, NS, BLKN], bf16, tag="ot", name=f"ot_{blk}")
                last = blk == NBLK - 1
                if last:
                    otl = op.tile([KWIN, BLKN], bf16, tag="otl", name="otl")
                else:
                    otl = None

                def evac(s, n=n, pss=pss, obt=obt, otl=otl, last=last):
                    # alternate Vector/Scalar so PSUM banks recycle 2x faster
                    eng = nc.vector.tensor_copy if s % 2 == 0 else nc.scalar.copy
                    if last and s == NS - 1:
                        eng(out=otl[:, 0:n], in_=pss[s])
                    else:
                        eng(out=obt[:, s, 0:n], in_=pss[s])

                if blk == 0:
                    # tile-major: each arriving x tile feeds every set using
                    # it, so the PE starts as soon as tile 0 lands.
                    for t in range(NTILE):
                        for s in range(NS):
                            tl = t - (1 if s >= 3 else 0)
                            if 0 <= tl < TPS:
                                nc.tensor.matmul(
                                    pss[s],
                                    wt[:, (s * TPS + tl) * 128:(s * TPS + tl + 1) * 128],
                                    xf[:, t * BLKN:t * BLKN + n],
                                    start=(tl == 0),
                                    stop=(tl == TPS - 1),
                                    skip_group_check=True,
                                )
                    for s in range(NS):
                        evac(s)
                else:
                    for s in range(NS):
                        g = 1 if s >= 3 else 0
                        for tl in range(TPS):
                            T = tl + g
                            nc.tensor.matmul(
                                pss[s],
                                wt[:, (s * TPS + tl) * 128:(s * TPS + tl + 1) * 128],
                                xf[:, T * BLKN:T * BLKN + n],
                                start=(tl == 0),
                                stop=(tl == TPS - 1),
                            )
                        evac(s)
                # one output DMA per block (128 descriptors of 6KB); the last
                # block splits off set 5 so the kernel tail only waits on 88KB
                if not last:
                    nc.sync.dma_start(out=osc[blk, :, :, 0:n], in_=obt[:, :, 0:n])
                else:
                    nc.sync.dma_start(out=osc[blk, :, 0:NS - 1, 0:n], in_=obt[:, 0:NS - 1, 0:n])
                    nc.sync.dma_start(out=osc[blk, :, NS - 1, 0:n], in_=otl[:, 0:n])
    nc.compile()
    return nc


def kernel(x: np.ndarray, templates: np.ndarray, onset_delays: np.ndarray) -> np.ndarray:
    global LAST_RESULT
    x = np.ascontiguousarray(x, dtype=np.float32)
    templates = np.asarray(templates, dtype=np.float32)
    onset_delays = np.asarray(onset_delays, dtype=np.float32)

    W = _build_weights(templates, onset_delays)
    Xsc = _build_xsc(x)                                   # (NCORES, NBLK, K, NTILE*BLKN)

    nc = _build_program()
    in_maps = [{"xsc": Xsc[c], "w": W} for c in range(NCORES)]
    res = run_bass_kernel_spmd(nc, in_maps, core_ids=list(range(NCORES)))
    LAST_RESULT = res

    osc = np.stack([np.asarray(r["osc"]) for r in res.results], axis=0)
    o = osc.astype(np.float32)                                 # (NCORES,NBLK,128,NS,BLKN)
    o = o.reshape(NCORES, NBLK, 8, E, NS, BLKN)                # core, blk, dd, e, s, n
    o = o.transpose(0, 1, 5, 4, 2, 3)                          # core, blk, n, s, dd, e
    o = np.ascontiguousarray(o).reshape(NCORES, CPAD, NS * 8 * E)
    o = o[:, :BPC * NCOLB, :].reshape(NCORES, BPC, NCOLB, NS, 8, E)
    o = o.reshape(B, NCOLB * Q, E)[:, :S, :]
    o = np.ascontiguousarray(o)
    o[:, S - 1, :] = 0.0                                   # reference zero-pads last column
    return o
